# revision 17
# baseline (speedup 1.0000x reference)
"""CrossCoder kernel for 8 Trainium2 NeuronCores (Bass/Tile, SPMD).

Math (reference):
    f     = relu(einsum('bld,ldf->bf', x, W_enc) + b_enc)     # [B, F]
    x_hat = einsum('bf,lfd->bld', f, W_dec) + b_dec           # [B, L, D]

Sharding: dict dim F=32768 split 8 ways (FL=4096 per core, tensor parallel
over latents). Each core computes its local f shard (encode) and the
partial decode sum over its latents. Cross-core reduction: bf16 AllToAll
+ on-core DVE sum (A2A moves half the bytes of a fp32 ReduceScatter and
has no CCE-reduce bottleneck). The host reassembles each core's output
slice.

Device layout is feature-major (contraction dim on SBUF partitions);
batch runs in two halves of 512 inside ONE TileContext. Matmul operands
(x, W_enc, W_dec, f) are bf16 (~4e-3 rel err vs 2e-2 tolerance); PSUM
stays fp32. The PE is clamped to 13/16 clock by a board GPIO throttle
(~262ns per N=512 matmul), so the kernel is issue-cadence-bound; every
other engine is arranged to never stall the PE:

- DMA issue costs ~0.6us of engine time each, so weights/x move as
  paired [128,1024] tiles (half the issues) on the sync HWDGE queue,
  everything else (partial stores, reduce loads, outputs) on the scalar
  HWDGE queue.
- Partials are stored p-major ([128, tiles, 512]) so each decode group's
  4 ld-tiles drain as ONE DMA; PSUM drains split vector/scalar.
- Half 0's partial (2MB bf16) is exchanged after half 0, reduce overlaps
  half 1. Half 1 is split into four (l,dg) groups, each AllToAll'd as
  soon as it is written; only the last ~512KB A2A + short reduce is
  exposed at the tail.
- x for half 1 prefetches during half 0's decode; half 0's x interleaves
  with first-group weights so the first matmul issues early.
"""

import numpy as np

B = 1024
L = 2
D = 1024
F = 32768
NCORES = 8
FL = F // NCORES      # 4096 latents per core
LD = L * D            # 2048
KT = LD // 128        # 16 encode k-tiles
KP = KT // 2          # 8 paired encode k-tiles
FT = FL // 128        # 32 f-tiles per core
FP2 = FT // 2         # 16 paired decode f-tiles
NB = 512              # matmul moving free dim
NH = 2                # batch halves

_CACHE = {}


def _build_nc():
    import concourse.bass as bass  # noqa: F401
    import concourse.tile as tile
    from concourse import bacc, mybir

    f32 = mybir.dt.float32
    bf16 = mybir.dt.bfloat16
    ADD = mybir.AluOpType.add
    RELU = mybir.ActivationFunctionType.Relu
    IDENT = mybir.ActivationFunctionType.Identity

    nc = bacc.Bacc()

    # paired tiles: [..., 128, 1024] = two [128,512] tiles side by side
    xT = nc.declare_dram_parameter("xT", [NH, KP, 128, 2 * NB], bf16, isOutput=False)
    w_enc = nc.declare_dram_parameter("w_enc", [FT // 4, KP, 128, 2 * NB], bf16, isOutput=False)
    w_dec = nc.declare_dram_parameter("w_dec", [L, 2, FP2, 128, 2 * NB], bf16, isOutput=False)
    b_enc = nc.declare_dram_parameter("b_enc", [128, FT], f32, isOutput=False)
    b_dec8 = nc.declare_dram_parameter("b_dec8", [128, KT], f32, isOutput=False)
    # out rows: [0:256] = h0 slice (acc0 [128,1024] flat); [256+64g:320+64g]
    # = h1 group g slice (acc1 [128,256] flat). See host remap below.
    out_sh = nc.declare_dram_parameter("out_sh", [512, NB], f32, isOutput=True)

    # partials p-major (bf16): [128, tiles, NB] so one DMA stores a group.
    # The final (l=1,dg=1) group is split into two 2-tile subgroups (5 and
    # 6) so the last exposed AllToAll is only 256KB.
    # h0's partial is exchanged per-l (two 1MB A2As) to halve the SDMA
    # contention window against half 1's weight stream
    parts0 = [nc.dram_tensor(f"partial0{l}", [128, 8, NB], bf16) for l in range(2)]
    _w1 = [4, 4, 4, 2, 1, 1]
    parts1 = [
        nc.dram_tensor(f"partial1{g}", [128, _w1[g], NB], bf16) for g in range(6)
    ]
    a2a0 = [nc.dram_tensor(f"a2a0{l}", [8, 16, 8, NB], bf16) for l in range(2)]
    a2a1 = [
        nc.dram_tensor(f"a2a1{g}", [8, 16, _w1[g], NB], bf16) for g in range(6)
    ]

    xT_a = xT.ap()
    w_enc_a = w_enc.ap()
    w_dec_a = w_dec.ap()
    rgroups = [list(range(NCORES))]

    with tile.TileContext(nc) as tc:
        with (
            tc.tile_pool(name="xp", bufs=1) as xp,
            tc.tile_pool(name="fp", bufs=1) as fp,
            tc.tile_pool(name="we", bufs=20) as we,
            tc.tile_pool(name="wd", bufs=26) as wd,
            tc.tile_pool(name="stg", bufs=2) as stg,
            tc.tile_pool(name="bias", bufs=1) as bias,
            tc.tile_pool(name="red", bufs=1) as red,
            tc.tile_pool(name="ps", bufs=8, space="PSUM") as ps,
        ):
            benc_t = None
            bdec_t = None

            x_tiles_h = [None, None]
            for h in range(NH):
                x_tiles = x_tiles_h[h]
                if x_tiles is None:
                    x_tiles = []
                    x_tiles_h[h] = x_tiles

                # ---- encode
                f_tiles = []
                for fg in range(FT // 4):
                    pss = [
                        ps.tile([128, NB], f32, tag="ps", name=f"pse{_j}")
                        for _j in range(4)
                    ]
                    for kp in range(KP):
                        if fg == 0 and h == 0:
                            # interleave x pairs with first-group weights
                            xt = xp.tile([128, 2 * NB], bf16, tag=f"x{h}_{kp}", name=f"x{h}_{kp}")
                            nc.sync.dma_start(out=xt, in_=xT_a[h, kp])
                            x_tiles.append(xt)
                        wt = we.tile([128, 2 * NB], bf16, tag="we", name="wet")
                        nc.sync.dma_start(out=wt, in_=w_enc_a[fg, kp])
                        for kin in range(2):
                            k = 2 * kp + kin
                            rhs = x_tiles[kp][:, kin * NB : (kin + 1) * NB]
                            for j in range(4):
                                nc.tensor.matmul(
                                    pss[j],
                                    wt[:, kin * NB + j * 128 : kin * NB + (j + 1) * 128],
                                    rhs,
                                    start=(k == 0),
                                    stop=(k == KT - 1),
                                )
                    if benc_t is None:
                        benc_t = bias.tile([128, FT], f32, name="benc")
                        nc.scalar.dma_start(out=benc_t, in_=b_enc.ap())
                    for j in range(4):
                        ft_idx = fg * 4 + j
                        ftile = fp.tile(
                            [128, NB], bf16, tag=f"f{ft_idx}", name=f"f{ft_idx}"
                        )
                        nc.scalar.activation(
                            ftile,
                            pss[j],
                            RELU,
                            bias=benc_t[:, ft_idx : ft_idx + 1],
                        )
                        f_tiles.append(ftile)

                if h == 0:
                    # prefetch half 1's x during half 0's decode
                    x_tiles_h[1] = []
                    for kp in range(KP):
                        xt = xp.tile([128, 2 * NB], bf16, tag=f"x1_{kp}", name=f"x1_{kp}")
                        nc.sync.dma_start(out=xt, in_=xT_a[1, kp])
                        x_tiles_h[1].append(xt)
                if bdec_t is None:
                    bdec_t = bias.tile([128, KT], f32, name="bdec")
                    nc.scalar.dma_start(out=bdec_t, in_=b_dec8.ap())

                # ---- decode
                if h == 0:
                    dgroups = [(l, dg, [0, 1, 2, 3], 2 * l + dg) for l in range(L) for dg in range(2)]
                else:
                    dgroups = [
                        (0, 0, [0, 1, 2, 3], 0),
                        (0, 1, [0, 1, 2, 3], 1),
                        (1, 0, [0, 1, 2, 3], 2),
                        (1, 1, [0, 1], 3),
                        (1, 1, [2], 4),
                        (1, 1, [3], 5),
                    ]
                for l, dg, js, g in dgroups:
                    T = len(js)
                    pss = [
                        ps.tile([128, NB], f32, tag="ps", name=f"psd{_j}")
                        for _j in range(T)
                    ]
                    for fkp in range(FP2):
                        wt = wd.tile([128, 2 * NB], bf16, tag="wd", name="wdt")
                        nc.sync.dma_start(out=wt, in_=w_dec_a[l, dg, fkp])
                        for kin in range(2):
                            fk = 2 * fkp + kin
                            for ji, j in enumerate(js):
                                nc.tensor.matmul(
                                    pss[ji],
                                    wt[:, kin * NB + j * 128 : kin * NB + (j + 1) * 128],
                                    f_tiles[fk],
                                    start=(fk == 0),
                                    stop=(fk == FT - 1),
                                )
                    # drain psum banks into one [128, T*NB] staging tile
                    # (split vector/scalar), then ONE p-major store
                    stb = stg.tile([128, T * NB], bf16, tag=f"st{T}", name="st")
                    for ji, j in enumerate(js):
                        ld_t = l * 8 + dg * 4 + j
                        dst = stb[:, ji * NB : (ji + 1) * NB]
                        if ji < T // 2:
                            nc.vector.tensor_scalar_add(
                                dst, pss[ji], bdec_t[:, ld_t : ld_t + 1]
                            )
                        else:
                            nc.scalar.activation(
                                dst, pss[ji], IDENT,
                                bias=bdec_t[:, ld_t : ld_t + 1],
                            )
                    if h == 0:
                        base = dg * 4
                        nc.scalar.dma_start(
                            out=parts0[l].ap()[:, base : base + 4, :], in_=stb
                        )
                        if dg == 1:
                            # this l's 8-tile partial (1MB) is complete →
                            # exchange now; reduce overlaps later compute
                            nc.gpsimd.collective_compute(
                                "AllToAll",
                                mybir.AluOpType.bypass,
                                ins=[parts0[l][:]],
                                outs=[a2a0[l][:]],
                                replica_groups=rgroups,
                            )
                            r0 = red.tile([128, 8 * NB], bf16, tag="red0", name="red0")
                            for jj in range(8):
                                eng = nc.sync if jj % 2 == 0 else nc.scalar
                                eng.dma_start(
                                    out=r0[:, jj * NB : (jj + 1) * NB],
                                    in_=a2a0[l].ap()[jj],
                                )
                            acc0 = red.tile([128, NB], f32, tag=f"acc0_{l}", name=f"acc0_{l}")
                            nc.vector.tensor_tensor(
                                acc0, r0[:, 0:NB], r0[:, NB : 2 * NB], ADD
                            )
                            for jj in range(2, 8):
                                nc.vector.tensor_tensor(
                                    acc0, acc0, r0[:, jj * NB : (jj + 1) * NB], ADD
                                )
                            nc.scalar.dma_start(
                                out=out_sh.ap()[128 * l : 128 * l + 128], in_=acc0
                            )
                    else:
                        cw = 64 * T  # per-chunk cols in the [128, ...] view
                        ro = 256 + [0, 64, 128, 192, 224, 240][g]
                        nc.scalar.dma_start(out=parts1[g].ap()[:], in_=stb)
                        # group complete → exchange + on-core reduce
                        nc.gpsimd.collective_compute(
                            "AllToAll",
                            mybir.AluOpType.bypass,
                            ins=[parts1[g][:]],
                            outs=[a2a1[g][:]],
                            replica_groups=rgroups,
                        )
                        r1 = red.tile([128, 8 * cw], bf16, tag=f"red1_{T}", name="red1")
                        for jj in range(8):
                            eng = nc.sync if jj % 2 == 0 else nc.scalar
                            eng.dma_start(
                                out=r1[:, jj * cw : (jj + 1) * cw],
                                in_=a2a1[g].ap()[jj],
                            )
                        acc1 = red.tile([128, cw], f32, tag=f"acc1_{g}", name=f"acc1_{g}")
                        nc.vector.tensor_tensor(
                            acc1, r1[:, 0:cw], r1[:, cw : 2 * cw], ADD
                        )
                        for jj in range(2, 8):
                            nc.vector.tensor_tensor(
                                acc1, acc1, r1[:, jj * cw : (jj + 1) * cw], ADD
                            )
                        nc.scalar.dma_start(
                            out=out_sh.ap()[ro : ro + 16 * T],
                            in_=acc1,
                        )
    nc.finalize()
    return nc


def _get_nc():
    if "nc" not in _CACHE:
        _CACHE["nc"] = _build_nc()
    return _CACHE["nc"]


def kernel(x, W_enc, b_enc, W_dec, b_dec):
    import ml_dtypes
    from concourse.bass_utils import run_bass_kernel_spmd

    bf16 = ml_dtypes.bfloat16
    x = np.asarray(x, dtype=np.float32)
    W_enc = np.asarray(W_enc, dtype=np.float32)
    b_enc = np.asarray(b_enc, dtype=np.float32)
    W_dec = np.asarray(W_dec, dtype=np.float32)
    b_dec = np.asarray(b_dec, dtype=np.float32)

    nc = _get_nc()

    # xT rows = x.reshape(B,LD).T; tile k holds rows k*128..k*128+128,
    # cols h*512..h*512+512; pair kp packs tiles {2kp, 2kp+1} side by side
    xTf = x.reshape(B, LD).T.reshape(KT, 128, NH, NB)          # [k,p,h,c]
    xT = np.ascontiguousarray(
        xTf.reshape(KP, 2, 128, NH, NB).transpose(3, 0, 2, 1, 4).reshape(NH, KP, 128, 2 * NB)
    ).astype(bf16)
    w_enc_flat = W_enc.reshape(LD, F)
    bdec8 = np.ascontiguousarray(
        (b_dec.reshape(LD) / NCORES).astype(np.float32).reshape(KT, 128).T
    )

    in_maps = []
    for i in range(NCORES):
        fsl = slice(i * FL, (i + 1) * FL)
        # [k, p, fg, c] -> pairs over k -> [fg, kp, p, 2c]
        we_t = w_enc_flat[:, fsl].reshape(KT, 128, FT // 4, NB)
        we_blk = np.ascontiguousarray(
            we_t.reshape(KP, 2, 128, FT // 4, NB).transpose(3, 0, 2, 1, 4).reshape(FT // 4, KP, 128, 2 * NB)
        ).astype(bf16)
        # W_dec[l, f, d]: tile (l, dg, fk) = [128 f-rows, 512 d-cols];
        # pair fkp packs {2fkp, 2fkp+1} side by side
        wd_t = W_dec[:, fsl, :].reshape(L, FT, 128, 2, NB)     # [l,fk,p,dg,c]
        wd_blk = np.ascontiguousarray(
            wd_t.reshape(L, FP2, 2, 128, 2, NB).transpose(0, 4, 1, 3, 2, 5).reshape(L, 2, FP2, 128, 2 * NB)
        ).astype(bf16)
        in_maps.append(
            {
                "xT": xT,
                "w_enc": we_blk,
                "w_dec": wd_blk,
                "b_enc": np.ascontiguousarray(b_enc[fsl].reshape(FT, 128).T),
                "b_dec8": bdec8,
            }
        )

    res = run_bass_kernel_spmd(nc, in_maps, list(range(NCORES)))
    _CACHE["last_res"] = res

    # Host reassembly. Partials are p-major [128p, T tiles, 512c]; the A2A
    # hands core i the flat chunk = partitions 16i..16i+16 of every tile.
    #   h0 (T=16): acc0 flat = [16pp, 16t, 512c] -> ld row t*128+16i+pp, col c
    #   h1 group g=(l,dg) (T=4): acc1 flat = [16pp, 4t, 512c]
    #       -> ld row (l*8+dg*4+t)*128 + 16i + pp, col 512+c
    xhatT = np.empty((LD, B), dtype=np.float32)
    xv = xhatT.reshape(KT, 128, B)
    for i in range(NCORES):
        arr = res.results[i]["out_sh"]  # [512, NB] fp32
        for l in range(2):
            h0 = arr[128 * l : 128 * l + 128].reshape(16, 8, NB).transpose(1, 0, 2)
            xv[8 * l : 8 * l + 8, 16 * i : 16 * i + 16, 0:NB] = h0
        # h1 groups: (l, dg, first ld-tile in group, T, row offset)
        for t0, T, ro in ((0, 4, 256), (4, 4, 320), (8, 4, 384), (12, 2, 448), (14, 1, 480), (15, 1, 496)):
            ch = arr[ro : ro + 16 * T].reshape(16, T, NB).transpose(1, 0, 2)
            xv[t0 : t0 + T, 16 * i : 16 * i + 16, NB : 2 * NB] = ch
    return np.ascontiguousarray(xhatT.T).reshape(B, L, D).astype(np.float32)


# revision 18
# speedup vs baseline: 1.0450x; 1.0450x over previous
"""CrossCoder kernel for 8 Trainium2 NeuronCores (Bass/Tile, SPMD).

Math (reference):
    f     = relu(einsum('bld,ldf->bf', x, W_enc) + b_enc)     # [B, F]
    x_hat = einsum('bf,lfd->bld', f, W_dec) + b_dec           # [B, L, D]

Sharding: dict dim F=32768 split 8 ways (FL=4096 per core, tensor parallel
over latents). Each core computes its local f shard (encode) and the
partial decode sum over its latents. Cross-core reduction: bf16 AllToAll
+ on-core DVE sum (A2A moves half the bytes of a fp32 ReduceScatter and
has no CCE-reduce bottleneck). The host reassembles each core's output
slice.

Device layout is feature-major (contraction dim on SBUF partitions);
batch runs in two halves of 512 inside ONE TileContext. Matmul operands
(x, W_enc, W_dec, f) are bf16 (~4e-3 rel err vs 2e-2 tolerance); PSUM
stays fp32. The PE is clamped to 13/16 clock by a board GPIO throttle
(~262ns per N=512 matmul), so the kernel is issue-cadence-bound; every
other engine is arranged to never stall the PE:

- DMA issue costs ~0.6us of engine time each, so weights/x move as
  paired [128,1024] tiles (half the issues) on the sync HWDGE queue,
  everything else (partial stores, reduce loads, outputs) on the scalar
  HWDGE queue.
- Partials are stored p-major ([128, tiles, 512]) so each decode group's
  4 ld-tiles drain as ONE DMA; PSUM drains split vector/scalar.
- Half 0's partial (2MB bf16) is exchanged after half 0, reduce overlaps
  half 1. Half 1 is split into four (l,dg) groups, each AllToAll'd as
  soon as it is written; only the last ~512KB A2A + short reduce is
  exposed at the tail.
- x for half 1 prefetches during half 0's decode; half 0's x interleaves
  with first-group weights so the first matmul issues early.
"""

import numpy as np

B = 1024
L = 2
D = 1024
F = 32768
NCORES = 8
FL = F // NCORES      # 4096 latents per core
LD = L * D            # 2048
KT = LD // 128        # 16 encode k-tiles
KP = KT // 2          # 8 paired encode k-tiles
FT = FL // 128        # 32 f-tiles per core
FP2 = FT // 2         # 16 paired decode f-tiles
NB = 512              # matmul moving free dim
NH = 2                # batch halves

_CACHE = {}


def _build_nc():
    import concourse.bass as bass  # noqa: F401
    import concourse.tile as tile
    from concourse import bacc, mybir

    f32 = mybir.dt.float32
    bf16 = mybir.dt.bfloat16
    ADD = mybir.AluOpType.add
    RELU = mybir.ActivationFunctionType.Relu
    IDENT = mybir.ActivationFunctionType.Identity

    nc = bacc.Bacc()

    # paired tiles: [..., 128, 1024] = two [128,512] tiles side by side
    xT = nc.declare_dram_parameter("xT", [NH, KP, 128, 2 * NB], bf16, isOutput=False)
    w_enc = nc.declare_dram_parameter("w_enc", [FT // 4, KP, 128, 2 * NB], bf16, isOutput=False)
    w_dec = nc.declare_dram_parameter("w_dec", [L, 2, FP2, 128, 2 * NB], bf16, isOutput=False)
    b_enc = nc.declare_dram_parameter("b_enc", [128, FT], f32, isOutput=False)
    b_dec8 = nc.declare_dram_parameter("b_dec8", [128, KT], f32, isOutput=False)
    # out rows: [0:256] = h0 slice (acc0 [128,1024] flat); [256+64g:320+64g]
    # = h1 group g slice (acc1 [128,256] flat). See host remap below.
    out_sh = nc.declare_dram_parameter("out_sh", [512, NB], f32, isOutput=True)

    # partials p-major (bf16): [128, tiles, NB] so one DMA stores a group.
    # The final (l=1,dg=1) group is split into two 2-tile subgroups (5 and
    # 6) so the last exposed AllToAll is only 256KB.
    # h0's partial is exchanged per-l (two 1MB A2As) to halve the SDMA
    # contention window against half 1's weight stream
    parts0 = [nc.dram_tensor(f"partial0{l}", [128, 8, NB], bf16) for l in range(2)]
    parts1 = [
        nc.dram_tensor(f"partial1{g}", [128, 4 if g < 3 else 2, NB], bf16)
        for g in range(5)
    ]
    a2a0 = [nc.dram_tensor(f"a2a0{l}", [8, 16, 8, NB], bf16) for l in range(2)]
    a2a1 = [
        nc.dram_tensor(f"a2a1{g}", [8, 16, 4 if g < 3 else 2, NB], bf16)
        for g in range(5)
    ]

    xT_a = xT.ap()
    w_enc_a = w_enc.ap()
    w_dec_a = w_dec.ap()
    rgroups = [list(range(NCORES))]

    with tile.TileContext(nc) as tc:
        with (
            tc.tile_pool(name="xp", bufs=1) as xp,
            tc.tile_pool(name="fp", bufs=1) as fp,
            tc.tile_pool(name="we", bufs=20) as we,
            tc.tile_pool(name="wd", bufs=26) as wd,
            tc.tile_pool(name="stg", bufs=2) as stg,
            tc.tile_pool(name="bias", bufs=1) as bias,
            tc.tile_pool(name="red", bufs=1) as red,
            tc.tile_pool(name="ps", bufs=8, space="PSUM") as ps,
        ):
            benc_t = None
            bdec_t = None

            x_tiles_h = [None, None]
            for h in range(NH):
                x_tiles = x_tiles_h[h]
                if x_tiles is None:
                    x_tiles = []
                    x_tiles_h[h] = x_tiles

                # ---- encode
                f_tiles = []
                for fg in range(FT // 4):
                    pss = [
                        ps.tile([128, NB], f32, tag="ps", name=f"pse{_j}")
                        for _j in range(4)
                    ]
                    for kp in range(KP):
                        if fg == 0 and h == 0:
                            # interleave x pairs with first-group weights
                            xt = xp.tile([128, 2 * NB], bf16, tag=f"x{h}_{kp}", name=f"x{h}_{kp}")
                            nc.sync.dma_start(out=xt, in_=xT_a[h, kp])
                            x_tiles.append(xt)
                        wt = we.tile([128, 2 * NB], bf16, tag="we", name="wet")
                        nc.sync.dma_start(out=wt, in_=w_enc_a[fg, kp])
                        for kin in range(2):
                            k = 2 * kp + kin
                            rhs = x_tiles[kp][:, kin * NB : (kin + 1) * NB]
                            for j in range(4):
                                nc.tensor.matmul(
                                    pss[j],
                                    wt[:, kin * NB + j * 128 : kin * NB + (j + 1) * 128],
                                    rhs,
                                    start=(k == 0),
                                    stop=(k == KT - 1),
                                )
                    if benc_t is None:
                        benc_t = bias.tile([128, FT], f32, name="benc")
                        nc.scalar.dma_start(out=benc_t, in_=b_enc.ap())
                    for j in range(4):
                        ft_idx = fg * 4 + j
                        ftile = fp.tile(
                            [128, NB], bf16, tag=f"f{ft_idx}", name=f"f{ft_idx}"
                        )
                        nc.scalar.activation(
                            ftile,
                            pss[j],
                            RELU,
                            bias=benc_t[:, ft_idx : ft_idx + 1],
                        )
                        f_tiles.append(ftile)

                if h == 0:
                    # prefetch half 1's x during half 0's decode
                    x_tiles_h[1] = []
                    for kp in range(KP):
                        xt = xp.tile([128, 2 * NB], bf16, tag=f"x1_{kp}", name=f"x1_{kp}")
                        nc.sync.dma_start(out=xt, in_=xT_a[1, kp])
                        x_tiles_h[1].append(xt)
                if bdec_t is None:
                    bdec_t = bias.tile([128, KT], f32, name="bdec")
                    nc.scalar.dma_start(out=bdec_t, in_=b_dec8.ap())

                # ---- decode
                if h == 0:
                    dgroups = [(l, dg, [0, 1, 2, 3], 2 * l + dg) for l in range(L) for dg in range(2)]
                else:
                    dgroups = [
                        (0, 0, [0, 1, 2, 3], 0),
                        (0, 1, [0, 1, 2, 3], 1),
                        (1, 0, [0, 1, 2, 3], 2),
                        (1, 1, [0, 1], 3),
                        (1, 1, [2, 3], 4),
                    ]
                for l, dg, js, g in dgroups:
                    T = len(js)
                    pss = [
                        ps.tile([128, NB], f32, tag="ps", name=f"psd{_j}")
                        for _j in range(T)
                    ]
                    for fkp in range(FP2):
                        wt = wd.tile([128, 2 * NB], bf16, tag="wd", name="wdt")
                        nc.sync.dma_start(out=wt, in_=w_dec_a[l, dg, fkp])
                        for kin in range(2):
                            fk = 2 * fkp + kin
                            for ji, j in enumerate(js):
                                nc.tensor.matmul(
                                    pss[ji],
                                    wt[:, kin * NB + j * 128 : kin * NB + (j + 1) * 128],
                                    f_tiles[fk],
                                    start=(fk == 0),
                                    stop=(fk == FT - 1),
                                )
                    # drain psum banks into one [128, T*NB] staging tile
                    # (split vector/scalar), then ONE p-major store
                    stb = stg.tile([128, T * NB], bf16, tag=f"st{T}", name="st")
                    for ji, j in enumerate(js):
                        ld_t = l * 8 + dg * 4 + j
                        dst = stb[:, ji * NB : (ji + 1) * NB]
                        if ji < T // 2:
                            nc.vector.tensor_scalar_add(
                                dst, pss[ji], bdec_t[:, ld_t : ld_t + 1]
                            )
                        else:
                            nc.scalar.activation(
                                dst, pss[ji], IDENT,
                                bias=bdec_t[:, ld_t : ld_t + 1],
                            )
                    if h == 0:
                        base = dg * 4
                        nc.scalar.dma_start(
                            out=parts0[l].ap()[:, base : base + 4, :], in_=stb
                        )
                        if dg == 1:
                            # this l's 8-tile partial (1MB) is complete →
                            # exchange now; reduce overlaps later compute
                            nc.gpsimd.collective_compute(
                                "AllToAll",
                                mybir.AluOpType.bypass,
                                ins=[parts0[l][:]],
                                outs=[a2a0[l][:]],
                                replica_groups=rgroups,
                            )
                            r0 = red.tile([128, 8 * NB], bf16, tag="red0", name="red0")
                            for jj in range(8):
                                eng = nc.sync if jj % 2 == 0 else nc.scalar
                                eng.dma_start(
                                    out=r0[:, jj * NB : (jj + 1) * NB],
                                    in_=a2a0[l].ap()[jj],
                                )
                            acc0 = red.tile([128, NB], f32, tag=f"acc0_{l}", name=f"acc0_{l}")
                            nc.vector.tensor_tensor(
                                acc0, r0[:, 0:NB], r0[:, NB : 2 * NB], ADD
                            )
                            for jj in range(2, 8):
                                nc.vector.tensor_tensor(
                                    acc0, acc0, r0[:, jj * NB : (jj + 1) * NB], ADD
                                )
                            nc.scalar.dma_start(
                                out=out_sh.ap()[128 * l : 128 * l + 128], in_=acc0
                            )
                    else:
                        cw = 64 * T  # per-chunk cols in the [128, ...] view
                        ro = 256 + [0, 64, 128, 192, 224][g]
                        nc.scalar.dma_start(out=parts1[g].ap()[:], in_=stb)
                        # group complete → exchange + on-core reduce
                        nc.gpsimd.collective_compute(
                            "AllToAll",
                            mybir.AluOpType.bypass,
                            ins=[parts1[g][:]],
                            outs=[a2a1[g][:]],
                            replica_groups=rgroups,
                        )
                        r1 = red.tile([128, 8 * cw], bf16, tag=f"red1_{T}", name="red1")
                        for jj in range(8):
                            eng = nc.sync if jj % 2 == 0 else nc.scalar
                            eng.dma_start(
                                out=r1[:, jj * cw : (jj + 1) * cw],
                                in_=a2a1[g].ap()[jj],
                            )
                        acc1 = red.tile([128, cw], f32, tag=f"acc1_{g}", name=f"acc1_{g}")
                        nc.vector.tensor_tensor(
                            acc1, r1[:, 0:cw], r1[:, cw : 2 * cw], ADD
                        )
                        for jj in range(2, 8):
                            nc.vector.tensor_tensor(
                                acc1, acc1, r1[:, jj * cw : (jj + 1) * cw], ADD
                            )
                        nc.scalar.dma_start(
                            out=out_sh.ap()[ro : ro + 16 * T],
                            in_=acc1,
                        )
    nc.finalize()
    return nc


def _get_nc():
    if "nc" not in _CACHE:
        _CACHE["nc"] = _build_nc()
    return _CACHE["nc"]


def kernel(x, W_enc, b_enc, W_dec, b_dec):
    import ml_dtypes
    from concourse.bass_utils import run_bass_kernel_spmd

    bf16 = ml_dtypes.bfloat16
    x = np.asarray(x, dtype=np.float32)
    W_enc = np.asarray(W_enc, dtype=np.float32)
    b_enc = np.asarray(b_enc, dtype=np.float32)
    W_dec = np.asarray(W_dec, dtype=np.float32)
    b_dec = np.asarray(b_dec, dtype=np.float32)

    nc = _get_nc()

    # xT rows = x.reshape(B,LD).T; tile k holds rows k*128..k*128+128,
    # cols h*512..h*512+512; pair kp packs tiles {2kp, 2kp+1} side by side
    xTf = x.reshape(B, LD).T.reshape(KT, 128, NH, NB)          # [k,p,h,c]
    xT = np.ascontiguousarray(
        xTf.reshape(KP, 2, 128, NH, NB).transpose(3, 0, 2, 1, 4).reshape(NH, KP, 128, 2 * NB)
    ).astype(bf16)
    w_enc_flat = W_enc.reshape(LD, F)
    bdec8 = np.ascontiguousarray(
        (b_dec.reshape(LD) / NCORES).astype(np.float32).reshape(KT, 128).T
    )

    in_maps = []
    for i in range(NCORES):
        fsl = slice(i * FL, (i + 1) * FL)
        # [k, p, fg, c] -> pairs over k -> [fg, kp, p, 2c]
        we_t = w_enc_flat[:, fsl].reshape(KT, 128, FT // 4, NB)
        we_blk = np.ascontiguousarray(
            we_t.reshape(KP, 2, 128, FT // 4, NB).transpose(3, 0, 2, 1, 4).reshape(FT // 4, KP, 128, 2 * NB)
        ).astype(bf16)
        # W_dec[l, f, d]: tile (l, dg, fk) = [128 f-rows, 512 d-cols];
        # pair fkp packs {2fkp, 2fkp+1} side by side
        wd_t = W_dec[:, fsl, :].reshape(L, FT, 128, 2, NB)     # [l,fk,p,dg,c]
        wd_blk = np.ascontiguousarray(
            wd_t.reshape(L, FP2, 2, 128, 2, NB).transpose(0, 4, 1, 3, 2, 5).reshape(L, 2, FP2, 128, 2 * NB)
        ).astype(bf16)
        in_maps.append(
            {
                "xT": xT,
                "w_enc": we_blk,
                "w_dec": wd_blk,
                "b_enc": np.ascontiguousarray(b_enc[fsl].reshape(FT, 128).T),
                "b_dec8": bdec8,
            }
        )

    res = run_bass_kernel_spmd(nc, in_maps, list(range(NCORES)))
    _CACHE["last_res"] = res

    # Host reassembly. Partials are p-major [128p, T tiles, 512c]; the A2A
    # hands core i the flat chunk = partitions 16i..16i+16 of every tile.
    #   h0 (T=16): acc0 flat = [16pp, 16t, 512c] -> ld row t*128+16i+pp, col c
    #   h1 group g=(l,dg) (T=4): acc1 flat = [16pp, 4t, 512c]
    #       -> ld row (l*8+dg*4+t)*128 + 16i + pp, col 512+c
    xhatT = np.empty((LD, B), dtype=np.float32)
    xv = xhatT.reshape(KT, 128, B)
    for i in range(NCORES):
        arr = res.results[i]["out_sh"]  # [512, NB] fp32
        for l in range(2):
            h0 = arr[128 * l : 128 * l + 128].reshape(16, 8, NB).transpose(1, 0, 2)
            xv[8 * l : 8 * l + 8, 16 * i : 16 * i + 16, 0:NB] = h0
        # h1 groups: (l, dg, first ld-tile in group, T, row offset)
        for t0, T, ro in ((0, 4, 256), (4, 4, 320), (8, 4, 384), (12, 2, 448), (14, 2, 480)):
            ch = arr[ro : ro + 16 * T].reshape(16, T, NB).transpose(1, 0, 2)
            xv[t0 : t0 + T, 16 * i : 16 * i + 16, NB : 2 * NB] = ch
    return np.ascontiguousarray(xhatT.T).reshape(B, L, D).astype(np.float32)
